# revision 12
# baseline (speedup 1.0000x reference)
"""BitLinear (ternary-weight linear) kernel for Trainium2, 8 NeuronCores.

Computation:  out = x @ (w_ternary * scale)^T
  where scale = max(mean(|weight|), 1e-5)
        w_ternary = clip(round(weight / scale), -1, 1)  in {-1, 0, 1}

Strategy:
  - Host: quantize the 4 MB weight (tiny, elementwise) and pre-transpose it
    to wT [in, out]; scale is folded into the kernel as an immediate.
  - Device (data-parallel over the batch dim, 1 batch row per core):
    out_b = x_b @ wT with ternary +/-1 weights, fp32r matmuls (full PE rate
    at free dim >= 256, ~13 mantissa bits so +/-1 weights are exact and x
    carries ~6e-5 relative rounding).
    Per 128-row block of x_b: DMA the natural [128, 1024] tile, PE-transpose
    its 8 column slices (contraction dim must sit on partitions), DVE-copy
    the transposed slices out of PSUM, then 16 accumulating matmuls
    (lhsT = xT tile, rhs = wT slice) produce PSUM [128 s, 1024 o] which the
    scalar engine copies out with the scale applied, and DMA stores.
"""

import numpy as np

B, S, IN, OUT = 8, 8192, 1024, 1024
N_CORES = 8
P = 128
S_BLOCKS = S // P  # 64
K_TILES = IN // P  # 8
EPS = 1e-5

_compiled = None


def _build():
    import concourse.bacc as bacc
    import concourse.mybir as mybir
    import concourse.tile as tile

    R = mybir.dt.float32r
    F32 = mybir.dt.float32

    nc = bacc.Bacc()
    x = nc.declare_dram_parameter("x", [S, IN], R, isOutput=False)
    wt = nc.declare_dram_parameter("wt", [IN, OUT], R, isOutput=False)
    ident = nc.declare_dram_parameter("ident", [P, P], R, isOutput=False)
    scale_t = nc.declare_dram_parameter("scale", [1, 1], F32, isOutput=False)
    out = nc.declare_dram_parameter("out", [S, OUT], F32, isOutput=True)

    with tile.TileContext(nc) as tc:
        with (
            tc.tile_pool(name="const", bufs=1) as constp,
            tc.tile_pool(name="xn", bufs=3) as xnp,
            tc.tile_pool(name="xt", bufs=6) as xtp,
            tc.tile_pool(name="outp", bufs=3) as outp,
            tc.tile_pool(name="pst", bufs=3, space="PSUM") as pst,
            tc.tile_pool(name="warm", bufs=1, space="PSUM") as warmp,
            tc.tile_pool(name="pso", bufs=4, space="PSUM") as pso,
        ):
            # Emit the x-tile loads first on the Sync (HWDGE) queue so the
            # first transposes aren't stuck behind the 4 MB weight DMA,
            # which goes on the GpSimd (SWDGE) queue instead.
            ident_sb = constp.tile([P, P], R)
            nc.sync.dma_start(out=ident_sb, in_=ident[:])

            xn_tiles = {}

            def load_xn(b):
                if b < S_BLOCKS and b not in xn_tiles:
                    t = xnp.tile([P, IN], R, tag="xn", name=f"xn_{b}")
                    nc.sync.dma_start(out=t, in_=x[b * P:(b + 1) * P, :])
                    xn_tiles[b] = t

            for b in range(2):
                load_xn(b)

            # full transposed ternary weight resident in SBUF: [128, k, 1024].
            # Split into per-k-tile DMAs on the GpSimd queue so the k=0 slice
            # lands in ~1.5us and block-0 matmuls don't wait for all 4 MB.
            wt_sb = constp.tile([P, K_TILES, OUT], R)
            wt_r = wt[:].rearrange("(a p) o -> p a o", p=P)
            for k in range(K_TILES):
                # scalar = ACT HWDGE ring, parallel to Sync's SP ring;
                # last two slices on the SWDGE ring to spread sem-lane load
                eng = nc.scalar if k < 6 else nc.gpsimd
                eng.dma_start(
                    out=wt_sb[:, k:k + 1, :], in_=wt_r[:, k:k + 1, :]
                )

            # Warm the PE (HAM clock gate) during the DMA-bound head with
            # dummy transposes of the identity tile; real work then starts
            # at the full 2.4 GHz instead of paying the 1.2 GHz cold ramp.
            warm_ps = warmp.tile([P, P], R, name="warm_ps")
            for _ in range(24):
                nc.tensor.transpose(warm_ps, ident_sb, ident_sb)

            # scale broadcast to all 128 partitions for the scaled copy
            # (after the weight DMAs: the 128-way replicated write is slow
            # and must not delay the k=0 weight slice)
            scale_sb = constp.tile([P, 1], F32)
            nc.gpsimd.dma_start(
                out=scale_sb, in_=scale_t[:].to_broadcast((P, 1))
            )

            for b in range(S_BLOCKS):
                xn_sb = xn_tiles.pop(b)
                load_xn(b + 2)

                # PE-transpose the 8 [128,128] column slices; pack 4 per
                # PSUM bank so 8 transposes only hold 2 banks.
                pts = [pst.tile([P, 4, P], R, tag="pst", name=f"pt{b}_{i}")
                       for i in range(2)]
                xts = [xtp.tile([P, 4, P], R, tag="xt4", name=f"xt{b}_{i}")
                       for i in range(2)]
                # copy each 4-pack on DVE as soon as its transposes finish,
                # so the first matmul of the block never waits on the copy
                for i in range(2):
                    for j in range(4):
                        k = 4 * i + j
                        nc.tensor.transpose(
                            pts[i][:, j, :],
                            xn_sb[:, k * P:(k + 1) * P],
                            ident_sb,
                        )
                    nc.vector.tensor_copy(xts[i], pts[i])

                po = [pso.tile([P, 512], F32, tag="pso", name=f"po{b}_{i}")
                      for i in range(2)]
                for k in range(K_TILES):
                    for h in range(2):
                        nc.tensor.matmul(
                            po[h],
                            lhsT=xts[k // 4][:, k % 4, :],
                            rhs=wt_sb[:, k, h * 512:(h + 1) * 512],
                            start=(k == 0),
                            stop=(k == K_TILES - 1),
                        )

                out_sb = outp.tile([P, OUT], F32)
                for h in range(2):
                    nc.scalar.activation(
                        out_sb[:, h * 512:(h + 1) * 512],
                        po[h],
                        mybir.ActivationFunctionType.Copy,
                        scale=scale_sb[:, 0:1],
                    )
                nc.sync.dma_start(
                    out=out[b * P:(b + 1) * P, :], in_=out_sb
                )
    nc.finalize()
    return nc


def _get_compiled():
    global _compiled
    if _compiled is None:
        _compiled = _build()
    return _compiled


def quantize_host(weight: np.ndarray):
    """Mirror of the reference ste_quantize, done on host in fp32.

    The mean is computed in float64 then rounded to fp32 so it tracks the
    true mean more closely than any fp32 summation order.
    """
    scale = np.float32(max(np.mean(np.abs(weight), dtype=np.float64), EPS))
    w_t = np.clip(np.round(weight / scale), -1.0, 1.0).astype(np.float32)
    return w_t, scale


def kernel(x: np.ndarray, weight: np.ndarray) -> np.ndarray:
    from concourse.bass_utils import run_bass_kernel_spmd

    assert x.shape == (B, S, IN) and weight.shape == (OUT, IN)
    w_t, scale = quantize_host(weight)
    wt_T = np.ascontiguousarray(w_t.T)  # [in, out]
    ident = np.eye(P, dtype=np.float32)
    scale_arr = np.array([[scale]], dtype=np.float32)

    nc = _get_compiled()
    in_maps = [
        {"x": np.ascontiguousarray(x[c]), "wt": wt_T, "ident": ident,
         "scale": scale_arr}
        for c in range(N_CORES)
    ]
    res = run_bass_kernel_spmd(nc, in_maps, core_ids=list(range(N_CORES)))
    return np.stack([res.results[c]["out"] for c in range(N_CORES)], axis=0)


# revision 19
# speedup vs baseline: 1.0474x; 1.0474x over previous
"""BitLinear (ternary-weight linear) kernel for Trainium2, 8 NeuronCores.

Computation:  out = x @ (w_ternary * scale)^T
  where scale = max(mean(|weight|), 1e-5)
        w_ternary = clip(round(weight / scale), -1, 1)  in {-1, 0, 1}

Strategy:
  - Host: quantize the 4 MB weight (tiny, elementwise) and pre-transpose it
    to wT [in, out]; scale is passed as a [1,1] tensor and applied by the
    scalar engine during the PSUM->SBUF output copy.
  - Device (data-parallel over the batch dim, 1 batch row per core):
    out_b = x_b @ wT with ternary +/-1 weights, fp32r matmuls (full PE rate
    at free dim >= 256, ~13 mantissa bits so +/-1 weights are exact and x
    carries ~6e-5 relative rounding).
    Per 128-row block of x_b: DMA the natural [128, 1024] tile, PE-transpose
    its 8 column slices (contraction dim must sit on partitions), DVE-copy
    the transposed slices out of PSUM, then 16 accumulating matmuls
    (lhsT = xT tile, rhs = wT slice) produce PSUM [128 s, 1024 o] which the
    scalar engine copies out with the scale applied, and DMA stores.
"""

import numpy as np

B, S, IN, OUT = 8, 8192, 1024, 1024
N_CORES = 8
P = 128
S_BLOCKS = S // P  # 64
K_TILES = IN // P  # 8
EPS = 1e-5

_compiled = None


def _build():
    import concourse.bacc as bacc
    import concourse.mybir as mybir
    import concourse.tile as tile

    R = mybir.dt.float32r
    F32 = mybir.dt.float32

    nc = bacc.Bacc()
    x = nc.declare_dram_parameter("x", [S, IN], R, isOutput=False)
    wt = nc.declare_dram_parameter("wt", [IN, OUT], R, isOutput=False)
    ident = nc.declare_dram_parameter("ident", [P, P], R, isOutput=False)
    scale_t = nc.declare_dram_parameter("scale", [1, 1], F32, isOutput=False)
    out = nc.declare_dram_parameter("out", [S, OUT], F32, isOutput=True)

    with tile.TileContext(nc) as tc:
        with (
            tc.tile_pool(name="const", bufs=1) as constp,
            tc.tile_pool(name="xn", bufs=3) as xnp,
            tc.tile_pool(name="xt", bufs=6) as xtp,
            tc.tile_pool(name="outp", bufs=3) as outp,
            tc.tile_pool(name="pst", bufs=4, space="PSUM") as pst,
            tc.tile_pool(name="pso", bufs=4, space="PSUM") as pso,
        ):
            ident_sb = constp.tile([P, P], R)
            nc.sync.dma_start(out=ident_sb, in_=ident[:])

            xn_tiles = {}

            def load_xn(b):
                if b < S_BLOCKS and b not in xn_tiles:
                    t = xnp.tile([P, IN], R, tag="xn", name=f"xn_{b}")
                    nc.sync.dma_start(out=t, in_=x[b * P:(b + 1) * P, :])
                    xn_tiles[b] = t

            load_xn(0)

            # Transposed ternary weight resident in SBUF: [128, k, 1024].
            # All startup DMAs go on the one Sync ring in priority order
            # (ident, x block 0, then weight k-slices interleaved with the
            # next x block) — a single ring drains strictly in order, so the
            # first transposes and first matmuls see their data earliest.
            wt_sb = constp.tile([P, K_TILES, OUT], R)
            wt_r = wt[:].rearrange("(a p) o -> p a o", p=P)
            for k in range(4):
                nc.sync.dma_start(
                    out=wt_sb[:, k:k + 1, :], in_=wt_r[:, k:k + 1, :]
                )
            load_xn(1)
            for k in range(4, K_TILES):
                nc.sync.dma_start(
                    out=wt_sb[:, k:k + 1, :], in_=wt_r[:, k:k + 1, :]
                )

            # scale broadcast to all 128 partitions for the scaled copy
            # (after the weight DMAs: the 128-way replicated write is slow
            # and must not delay the k=0 weight slice)
            scale_sb = constp.tile([P, 1], F32)
            nc.gpsimd.dma_start(
                out=scale_sb, in_=scale_t[:].to_broadcast((P, 1))
            )

            # Software-pipelined emission: the PE-transposes (+DVE copies)
            # for block b+1 are emitted BEFORE block b's matmuls, so the
            # copies complete during the 3.6us matmul phase and the next
            # block's first matmul never stalls on its transposed operand.
            def emit_transposes(b):
                # PE-transpose the 8 [128,128] column slices; pack 4 per
                # PSUM bank so 8 transposes only hold 2 banks.
                xn_sb = xn_tiles.pop(b)
                load_xn(b + 2)
                pts = [pst.tile([P, 4, P], R, tag="pst", name=f"pt{b}_{i}")
                       for i in range(2)]
                xts = [xtp.tile([P, 4, P], R, tag="xt4", name=f"xt{b}_{i}")
                       for i in range(2)]
                for i in range(2):
                    for j in range(4):
                        k = 4 * i + j
                        nc.tensor.transpose(
                            pts[i][:, j, :],
                            xn_sb[:, k * P:(k + 1) * P],
                            ident_sb,
                        )
                    nc.vector.tensor_copy(xts[i], pts[i])
                return xts

            xts_cur = emit_transposes(0)
            for b in range(S_BLOCKS):
                xts_next = (emit_transposes(b + 1)
                            if b + 1 < S_BLOCKS else None)

                # h-outer: finish the o-half-0 accumulation first so its
                # scaled copy + store overlap the o-half-1 matmuls; per-
                # element k order is unchanged, so numerics are identical.
                out_sb = outp.tile([P, OUT], F32)
                for h in range(2):
                    po_h = pso.tile([P, 512], F32, tag="pso",
                                    name=f"po{b}_{h}")
                    for k in range(K_TILES):
                        nc.tensor.matmul(
                            po_h,
                            lhsT=xts_cur[k // 4][:, k % 4, :],
                            rhs=wt_sb[:, k, h * 512:(h + 1) * 512],
                            start=(k == 0),
                            stop=(k == K_TILES - 1),
                        )
                    nc.scalar.activation(
                        out_sb[:, h * 512:(h + 1) * 512],
                        po_h,
                        mybir.ActivationFunctionType.Copy,
                        scale=scale_sb[:, 0:1],
                    )
                    nc.sync.dma_start(
                        out=out[b * P:(b + 1) * P,
                                h * 512:(h + 1) * 512],
                        in_=out_sb[:, h * 512:(h + 1) * 512],
                    )
                xts_cur = xts_next
    nc.finalize()
    return nc


def _get_compiled():
    global _compiled
    if _compiled is None:
        _compiled = _build()
    return _compiled


def quantize_host(weight: np.ndarray):
    """Mirror of the reference ste_quantize, done on host in fp32.

    The mean is computed in float64 then rounded to fp32 so it tracks the
    true mean more closely than any fp32 summation order.
    """
    scale = np.float32(max(np.mean(np.abs(weight), dtype=np.float64), EPS))
    w_t = np.clip(np.round(weight / scale), -1.0, 1.0).astype(np.float32)
    return w_t, scale


def kernel(x: np.ndarray, weight: np.ndarray) -> np.ndarray:
    from concourse.bass_utils import run_bass_kernel_spmd

    x = np.asarray(x, dtype=np.float32)
    weight = np.asarray(weight, dtype=np.float32)
    assert x.shape == (B, S, IN) and weight.shape == (OUT, IN)
    w_t, scale = quantize_host(weight)
    wt_T = np.ascontiguousarray(w_t.T)  # [in, out]
    ident = np.eye(P, dtype=np.float32)
    scale_arr = np.array([[scale]], dtype=np.float32)

    nc = _get_compiled()
    in_maps = [
        {"x": np.ascontiguousarray(x[c]), "wt": wt_T, "ident": ident,
         "scale": scale_arr}
        for c in range(N_CORES)
    ]
    res = run_bass_kernel_spmd(nc, in_maps, core_ids=list(range(N_CORES)))
    return np.stack([res.results[c]["out"] for c in range(N_CORES)], axis=0)

